# revision 24
# baseline (speedup 1.0000x reference)
"""Gemma attention (B=2, S=2048, HID=2048, H=8 q-heads, 1 KV head, D=256)
as a Bass/Tile SPMD kernel on 8 TRN2 NeuronCores.

Distribution (tensor-parallel over query heads):
  - core c owns query head c: wq/wo split along the head axis.
  - k/v projection is sharded over tokens (512 tokens/core), then
    AllGathered (k in transposed layout, v in natural layout; a ones
    column on v makes the softmax denominator fall out of the PV matmul).
  - softmax skips the max-subtraction (scores ~ N(0,1); exp is safe in
    fp32) and is computed on the transposed score layout so no transposes
    are needed before the PV matmul.
  - o_proj: per-head attention outputs (transposed [D, T]) are
    AllGathered quarter-by-quarter (pipelined behind attention) to form
    A.T = [H*D, T]; each core computes its own 256-column slice of the
    output, so no AllReduce is needed at all.

DMA ring notes: HWDGE FIFOs are per issuing engine (sync=SP, scalar=ACT)
and a DMA that waits on a collective blocks everything behind it on the
same ring. Streaming loads alternate between the SP and ACT rings for
bandwidth; collective bounce-buffer stores go on the idle GpSimd SWDGE;
collective-result loads go at points where their ring is already drained.

All matmuls run in bf16 with fp32 PSUM accumulation; RoPE cos/sin tables
are precomputed on the host from position_ids.
"""
import numpy as np
import ml_dtypes

import concourse.bass as bass
import concourse.mybir as mybir
import concourse.tile as tile
from concourse import bacc
from concourse.bass_utils import run_bass_kernel_spmd
from concourse.masks import make_identity

B, S, HID = 2, 2048, 2048
H, D = 8, 256
N_CORES = 8
T = B * S              # 4096 tokens total
SH = T // N_CORES      # 512 kv tokens per core
BASE = 10000.0
BF16 = mybir.dt.bfloat16
F32 = mybir.dt.float32
RG = [list(range(N_CORES))]
AF = mybir.ActivationFunctionType
_bf = ml_dtypes.bfloat16

KC = HID // 128        # 16 contraction chunks
SCALE = 1.0 / np.sqrt(D)


class _RingChain:
    """Chains DMA loads per HWDGE ring in emission order so the scheduler
    cannot hoist a collective-dependent load above the streaming loads
    (head-of-line blocking on the ring FIFO)."""

    def __init__(self, nc):
        self.engs = [nc.sync, nc.scalar]
        self.prev = [None, None]

    def dma(self, ring, dst, src):
        from concourse.tile_rust import add_dep_helper
        li = self.engs[ring].dma_start(dst, src)
        if self.prev[ring] is not None:
            add_dep_helper(li.ins, self.prev[ring].ins, sync=False,
                           reason="ring FIFO order")
        self.prev[ring] = li
        return li


def _attention_block(nc, psA, psB, psT, ptp, work, kt_sb, v_sb, q_sb, o_sb,
                     ident, b, tb):
    """Scores^T -> exp -> PV (ones-augmented) -> normalize -> transpose."""
    tq = b * S + tb * 512
    pt = ptp.tile([128, 16 * 512], BF16, tag="pt", name=f"pt{b}_{tb}")
    for uc in range(16):
        sp = psA.tile([128, 512], F32, tag="mm512", name=f"sp{b}_{tb}_{uc}")
        for dc in range(2):
            x = (b * 4 + uc // 4) * 2 + dc
            nc.tensor.matmul(
                sp[:],
                lhsT=kt_sb[:, x * 512 + (uc % 4) * 128:x * 512 + (uc % 4 + 1) * 128],
                rhs=q_sb[dc][:, tq:tq + 512],
                start=(dc == 0), stop=(dc == 1))
        nc.scalar.activation(pt[:, uc * 512:(uc + 1) * 512], sp[:],
                             AF.Exp, scale=float(SCALE))
    for ts in range(4):
        av = psB.tile([128, 257], F32, tag="acc", name=f"av{b}_{tb}_{ts}")
        for uc in range(16):
            nc.tensor.matmul(
                av[:],
                lhsT=pt[:, uc * 512 + ts * 128:uc * 512 + (ts + 1) * 128],
                rhs=v_sb[:, (b * 16 + uc) * 257:(b * 16 + uc + 1) * 257],
                start=(uc == 0), stop=(uc == 15))
        recip = work.tile([128, 1], F32, tag="recip", name=f"rc{b}_{tb}_{ts}")
        nc.vector.reciprocal(recip[:], av[:, 256:257])
        onat = work.tile([128, 256], BF16, tag="onat", name=f"on{b}_{tb}_{ts}")
        nc.scalar.activation(onat[:], av[:, 0:256], AF.Copy, scale=recip[:])
        for dcc in range(2):
            trp = psT.tile([128, 128], BF16, tag="tr", name=f"tr{b}_{tb}_{ts}_{dcc}")
            nc.tensor.transpose(trp[:], onat[:, dcc * 128:(dcc + 1) * 128], ident[:])
            nc.vector.tensor_copy(
                o_sb[b][dcc][:, tb * 512 + ts * 128:tb * 512 + (ts + 1) * 128],
                trp[:])


def _oag_start(nc, dram, o_sb, oag, tok0, width):
    """Store A^T for tokens [tok0, tok0+width) to DRAM and AllGather it."""
    b, off = tok0 // S, tok0 % S
    oin = dram.tile([256, width], BF16, name=f"oag_in{tok0}")
    oout = dram.tile([2048, width], BF16, addr_space="Shared",
                     name=f"oag_out{tok0}")
    for dcc in range(2):
        nc.gpsimd.dma_start(oin[dcc * 128:(dcc + 1) * 128, :],
                            o_sb[b][dcc][:, off:off + width])
    nc.gpsimd.collective_compute(
        "AllGather", mybir.AluOpType.bypass, replica_groups=RG,
        ins=[oin[:]], outs=[oout[:]])
    oag.append((oout, tok0, width))


def _oproj_piece(nc, psA, stB, work, wo_sb, out, oag, q, chain):
    """o_proj for one gathered A^T piece (width 512 or 1024 tokens)."""
    oout, tok0, width = oag[q]
    at = stB.tile([128, 16 * 1024], BF16, tag="at", name=f"at{q}")
    for ji in range(4):
        chain.dma(
            0,
            at[:, ji * width * 4:(ji + 1) * width * 4]
                .rearrange("p (x t) -> p x t", x=4),
            oout[ji * 512:(ji + 1) * 512, :].rearrange("(x p) t -> p x t", p=128))
    # transposed o_proj: out^T[c, t] = sum_j woT[j, c] * A^T[j, t] — N=512
    # moving dim, half the matmul instructions; host untransposes.
    for h in range(width // 512):
        for cc in range(2):
            op = psA.tile([128, 512], F32, tag="mm512", name=f"op{q}_{h}_{cc}")
            for jc in range(16):
                nc.tensor.matmul(
                    op[:],
                    lhsT=wo_sb[:, jc * 256 + cc * 128:jc * 256 + (cc + 1) * 128],
                    rhs=at[:, jc * width + h * 512:jc * width + (h + 1) * 512],
                    start=(jc == 0), stop=(jc == KC - 1))
            osb = work.tile([128, 512], F32, tag="osb", name=f"os{q}_{h}_{cc}")
            nc.scalar.copy(osb[:], op[:])
            col = tok0 + h * 512
            nc.scalar.dma_start(out[cc * 128:(cc + 1) * 128, col:col + 512], osb[:])


def _body(nc, tc, io):
    hsT, hskv = io["hsT"], io["hskv"]
    wq, wk, wv, wo = io["wq"], io["wk"], io["wv"], io["wo"]
    cosT, sinT = io["cosT"], io["sinT"]
    coskv, sinkv = io["coskv"], io["sinkv"]
    out = io["out"]

    with (
        tc.tile_pool(name="const", bufs=1) as constp,
        tc.tile_pool(name="pers", bufs=1) as pers,
        tc.tile_pool(name="work", bufs=2) as work,
        tc.tile_pool(name="dram", bufs=1, space="DRAM") as dram,
    ):
        ph3_cm = tc.tile_pool(name="ph3", bufs=1)
        ph3 = ph3_cm.__enter__()
        phcs_cm = tc.tile_pool(name="phcs", bufs=1)
        phcs = phcs_cm.__enter__()
        ph12_cm = tc.tile_pool(name="ph12", bufs=1)
        ph12 = ph12_cm.__enter__()
        psA_cm = tc.tile_pool(name="psA", bufs=4, space="PSUM")
        psA = psA_cm.__enter__()
        psB_cm = tc.tile_pool(name="psB", bufs=3, space="PSUM")
        psB = psB_cm.__enter__()
        psT_cm = tc.tile_pool(name="psT", bufs=1, space="PSUM")
        psT = psT_cm.__enter__()

        # ---- kv-critical loads first, split across both HWDGE rings ----
        chain = _RingChain(nc)
        wk_sb = constp.tile([128, KC * 256], BF16, name="wk_sb")
        wv_sb = constp.tile([128, KC * 256], BF16, name="wv_sb")
        hskv_sb = ph12.tile([128, KC * SH], BF16, name="hskv_sb")
        chain.dma(0, wk_sb[:, 0:2048], wk[:, 0:2048])
        chain.dma(1, wv_sb[:, 0:2048], wv[:, 0:2048])
        for h in range(4):
            chain.dma(h % 2, hskv_sb[:, h * 2048:(h + 1) * 2048],
                      hskv[:, h * 2048:(h + 1) * 2048])
        chain.dma(0, wk_sb[:, 2048:4096], wk[:, 2048:4096])
        chain.dma(1, wv_sb[:, 2048:4096], wv[:, 2048:4096])
        coskv_sb = constp.tile([128, SH], BF16, name="coskv_sb")
        chain.dma(1, coskv_sb[:], coskv[:])
        sinkv_sb = constp.tile([128, SH], BF16, name="sinkv_sb")
        chain.dma(1, sinkv_sb[:], sinkv[:])
        wq_sb = constp.tile([128, KC * 256], BF16, name="wq_sb")
        chain.dma(0, wq_sb[:], wq[:])
        cosT_sb = phcs.tile([128, T], BF16, name="cosT_sb")
        chain.dma(1, cosT_sb[:], cosT[:])
        sinT_sb = phcs.tile([128, T], BF16, name="sinT_sb")
        chain.dma(1, sinT_sb[:], sinT[:])
        ident = constp.tile([128, 128], BF16, name="ident")
        make_identity(nc, ident[:])

        # ---- DRAM comm buffers (k AG first so scores unblock earliest) ----
        kag_in = dram.tile([256, SH], BF16, name="kag_in")
        kag_out = dram.tile([256 * N_CORES, SH], BF16, addr_space="Shared",
                            name="kag_out")
        vag_in = dram.tile([SH, 257], BF16, name="vag_in")
        vag_out = dram.tile([T, 257], BF16, addr_space="Shared", name="vag_out")

        # ---- phase 1: kv projection on this core's 512 tokens ----
        kps = []
        for dc in range(2):
            kp = psA.tile([128, SH], F32, tag="mm512", name=f"kp{dc}")
            for kc in range(KC):
                nc.tensor.matmul(
                    kp[:],
                    lhsT=wk_sb[:, kc * 256 + dc * 128:kc * 256 + (dc + 1) * 128],
                    rhs=hskv_sb[:, kc * SH:(kc + 1) * SH],
                    start=(kc == 0), stop=(kc == KC - 1))
            kps.append(kp)
        for dc in range(2):
            ra = work.tile([128, SH], F32, tag="ropeA", name=f"kra{dc}")
            rb = work.tile([128, SH], F32, tag="ropeB", bufs=1, name=f"krb{dc}")
            kst = work.tile([128, SH], BF16, tag="kst", bufs=1, name=f"kst{dc}")
            if dc == 0:
                nc.vector.tensor_mul(ra[:], kps[0][:], coskv_sb[:])
                nc.vector.tensor_mul(rb[:], kps[1][:], sinkv_sb[:])
                nc.vector.tensor_sub(kst[:], ra[:], rb[:])
            else:
                nc.vector.tensor_mul(ra[:], kps[1][:], coskv_sb[:])
                nc.vector.tensor_mul(rb[:], kps[0][:], sinkv_sb[:])
                nc.vector.tensor_add(kst[:], ra[:], rb[:])
            nc.gpsimd.dma_start(kag_in[dc * 128:(dc + 1) * 128, :], kst[:])
        nc.gpsimd.collective_compute(
            "AllGather", mybir.AluOpType.bypass, replica_groups=RG,
            ins=[kag_in[:]], outs=[kag_out[:]])
        for uu in range(4):
            vp = psB.tile([128, 257], F32, tag="acc", name=f"vp{uu}")
            for kc in range(KC):
                nc.tensor.matmul(
                    vp[:, 0:256],
                    lhsT=hskv_sb[:, kc * SH + uu * 128:kc * SH + (uu + 1) * 128],
                    rhs=wv_sb[:, kc * 256:(kc + 1) * 256],
                    start=(kc == 0), stop=(kc == KC - 1))
            vst = work.tile([128, 257], BF16, tag="vst", bufs=1, name=f"vst{uu}")
            nc.scalar.copy(vst[:, 0:256], vp[:, 0:256])
            nc.vector.memset(vst[:, 256:257], 1.0)
            nc.gpsimd.dma_start(vag_in[uu * 128:(uu + 1) * 128, :], vst[:])
        nc.gpsimd.collective_compute(
            "AllGather", mybir.AluOpType.bypass, replica_groups=RG,
            ins=[vag_in[:]], outs=[vag_out[:]])

        # ---- phase 2: q projection + RoPE; batch-0 tiles also produce
        # LOCAL k/v for batch 0 (attention b0 then has no collective dep;
        # the kv AllGather only matters for batch 1, hiding the CC floor).
        q_sb = [ph3.tile([128, T], BF16, name=f"q{dc}_sb") for dc in range(2)]
        kt_sb = ph3.tile([128, 16 * 512], BF16, name="kt_sb")
        v_sb = ph3.tile([128, 32 * 257], BF16, name="v_sb")

        def q_tile(tb):
            hst = ph12.tile([128, KC * 512], BF16, tag="hst", bufs=4,
                            name=f"hst{tb}")
            chain.dma(
                tb % 2,
                hst.rearrange("p (x t) -> p x t", x=KC),
                hsT[:, tb * 512:(tb + 1) * 512].rearrange("(x p) t -> p x t", p=128))
            qps = []
            for dc in range(2):
                qp = psA.tile([128, 512], F32, tag="mm512", name=f"qp{tb}_{dc}")
                for kc in range(KC):
                    nc.tensor.matmul(
                        qp[:],
                        lhsT=wq_sb[:, kc * 256 + dc * 128:kc * 256 + (dc + 1) * 128],
                        rhs=hst[:, kc * 512:(kc + 1) * 512],
                        start=(kc == 0), stop=(kc == KC - 1))
                qps.append(qp)
            cs = cosT_sb[:, tb * 512:(tb + 1) * 512]
            sn = sinT_sb[:, tb * 512:(tb + 1) * 512]
            for dc in range(2):
                ra = work.tile([128, 512], F32, tag="ropeA", name=f"qra{tb}_{dc}")
                rb = work.tile([128, 512], F32, tag="ropeB", bufs=1, name=f"qrb{tb}_{dc}")
                if dc == 0:
                    nc.vector.tensor_mul(ra[:], qps[0][:], cs)
                    nc.vector.tensor_mul(rb[:], qps[1][:], sn)
                    nc.vector.tensor_sub(q_sb[0][:, tb * 512:(tb + 1) * 512], ra[:], rb[:])
                else:
                    nc.vector.tensor_mul(ra[:], qps[1][:], cs)
                    nc.vector.tensor_mul(rb[:], qps[0][:], sn)
                    nc.vector.tensor_add(q_sb[1][:, tb * 512:(tb + 1) * 512], ra[:], rb[:])
            if tb >= 4:
                return
            # local k^T for batch 0, u-block = tb
            kqs = []
            for dc in range(2):
                kq = psA.tile([128, 512], F32, tag="mm512", name=f"klo{tb}_{dc}")
                for kc in range(KC):
                    nc.tensor.matmul(
                        kq[:],
                        lhsT=wk_sb[:, kc * 256 + dc * 128:kc * 256 + (dc + 1) * 128],
                        rhs=hst[:, kc * 512:(kc + 1) * 512],
                        start=(kc == 0), stop=(kc == KC - 1))
                kqs.append(kq)
            for dc in range(2):
                x = tb * 2 + dc
                ra = work.tile([128, 512], F32, tag="ropeA", name=f"kla{tb}_{dc}")
                rb = work.tile([128, 512], F32, tag="ropeB", bufs=1, name=f"klb{tb}_{dc}")
                if dc == 0:
                    nc.vector.tensor_mul(ra[:], kqs[0][:], cs)
                    nc.vector.tensor_mul(rb[:], kqs[1][:], sn)
                    nc.vector.tensor_sub(kt_sb[:, x * 512:(x + 1) * 512], ra[:], rb[:])
                else:
                    nc.vector.tensor_mul(ra[:], kqs[1][:], cs)
                    nc.vector.tensor_mul(rb[:], kqs[0][:], sn)
                    nc.vector.tensor_add(kt_sb[:, x * 512:(x + 1) * 512], ra[:], rb[:])
            # local v for batch 0, u-chunks tb*4..tb*4+3
            for uu in range(4):
                vq = psB.tile([128, 257], F32, tag="acc", name=f"vlo{tb}_{uu}")
                for kc in range(KC):
                    nc.tensor.matmul(
                        vq[:, 0:256],
                        lhsT=hst[:, kc * 512 + uu * 128:kc * 512 + (uu + 1) * 128],
                        rhs=wv_sb[:, kc * 256:(kc + 1) * 256],
                        start=(kc == 0), stop=(kc == KC - 1))
                col = (tb * 4 + uu) * 257
                nc.scalar.copy(v_sb[:, col:col + 256], vq[:, 0:256])
                nc.vector.memset(v_sb[:, col + 256:col + 257], 1.0)

        for tb in range(T // 512):
            q_tile(tb)
        ph12_cm.__exit__(None, None, None)
        phcs_cm.__exit__(None, None, None)

        ptp_cm = tc.tile_pool(name="ptp", bufs=2)
        ptp = ptp_cm.__enter__()
        stB_cm = tc.tile_pool(name="stB", bufs=2)
        stB = stB_cm.__enter__()

        # wo needed from the first o_proj piece (~50% into the kernel)
        wo_sb = constp.tile([128, KC * 256], BF16, name="wo_sb")
        chain.dma(0, wo_sb[:], wo[:])

        # ---- phases 2b/3/4 interleaved ----
        o_sb = [[pers.tile([128, S], BF16, name=f"o{b}_{dcc}_sb")
                 for dcc in range(2)] for b in range(2)]
        oag = []
        ab = lambda b, tb: _attention_block(nc, psA, psB, psT, ptp, work, kt_sb,
                                            v_sb, q_sb, o_sb, ident, b, tb)
        op = lambda q: _oproj_piece(nc, psA, stB, work, wo_sb, out, oag, q,
                                    chain)
        # gathered k/v for batch 1 (ranks 4-7 only; chained on sync ring)
        for r in range(4, N_CORES):
            chain.dma(
                0, kt_sb[:, r * 1024:(r + 1) * 1024].rearrange("p (x u) -> p x u", x=2),
                kag_out[r * 256:(r + 1) * 256, :].rearrange("(x p) u -> p x u", p=128))
        for r in range(4, N_CORES):
            chain.dma(
                0, v_sb[:, r * 4 * 257:(r + 1) * 4 * 257].rearrange("p (x d) -> p x d", x=4),
                vag_out[r * 512:(r + 1) * 512, :].rearrange("(x p) d -> p x d", p=128))
        ab(0, 0); ab(0, 1)
        _oag_start(nc, dram, o_sb, oag, 0, 1024)
        ab(0, 2); ab(0, 3)
        _oag_start(nc, dram, o_sb, oag, 1024, 1024)
        op(0)
        ab(1, 0); ab(1, 1)
        _oag_start(nc, dram, o_sb, oag, 2048, 1024)
        op(1)
        ab(1, 2)
        _oag_start(nc, dram, o_sb, oag, 3072, 512)
        ab(1, 3)
        _oag_start(nc, dram, o_sb, oag, 3584, 512)
        op(2)
        op(3)
        op(4)

        stB_cm.__exit__(None, None, None)
        ptp_cm.__exit__(None, None, None)
        ph3_cm.__exit__(None, None, None)
        psT_cm.__exit__(None, None, None)
        psB_cm.__exit__(None, None, None)
        psA_cm.__exit__(None, None, None)


_NC_CACHE = {}


def _build():
    if "nc" in _NC_CACHE:
        return _NC_CACHE["nc"]
    nc = bacc.Bacc("TRN2", target_bir_lowering=False, debug=False,
                   enable_asserts=False, num_devices=N_CORES)
    io = {}
    io["hsT"] = nc.dram_tensor("hsT", [HID, T], BF16, kind="ExternalInput").ap()
    io["hskv"] = nc.dram_tensor("hskv", [128, KC * SH], BF16, kind="ExternalInput").ap()
    for w in ("wq", "wk", "wv", "wo"):
        io[w] = nc.dram_tensor(w, [128, KC * 256], BF16, kind="ExternalInput").ap()
    io["cosT"] = nc.dram_tensor("cosT", [128, T], BF16, kind="ExternalInput").ap()
    io["sinT"] = nc.dram_tensor("sinT", [128, T], BF16, kind="ExternalInput").ap()
    io["coskv"] = nc.dram_tensor("coskv", [128, SH], BF16, kind="ExternalInput").ap()
    io["sinkv"] = nc.dram_tensor("sinkv", [128, SH], BF16, kind="ExternalInput").ap()
    io["out"] = nc.dram_tensor("out", [256, T], F32, kind="ExternalOutput").ap()
    with tile.TileContext(nc) as tc:
        _body(nc, tc, io)
    nc.compile()
    _NC_CACHE["nc"] = nc
    return nc


def _tile_kxm(a):
    """[HID, M] -> [128, KC*M] with column block kc holding rows kc*128..+128."""
    hid, m = a.shape
    return np.ascontiguousarray(
        a.reshape(hid // 128, 128, m).transpose(1, 0, 2).reshape(128, -1))


def _prepare(hidden_states, position_ids, wq, wk, wv, wo):
    hs = np.asarray(hidden_states, dtype=np.float32).reshape(T, HID)
    hsT = np.ascontiguousarray(hs.T).astype(_bf)                 # [HID, T]

    inv_freq = 1.0 / (BASE ** (np.arange(0, D, 2, dtype=np.float64) / D))
    pos = np.asarray(position_ids).astype(np.float64).reshape(T)
    ang = inv_freq[:, None] * pos[None, :]                        # [128, T]
    cosT = np.cos(ang).astype(_bf)
    sinT = np.sin(ang).astype(_bf)

    wq = np.asarray(wq, dtype=np.float32)
    wk = np.asarray(wk, dtype=np.float32)
    wv = np.asarray(wv, dtype=np.float32)
    wo = np.asarray(wo, dtype=np.float32)
    wkT = _tile_kxm(wk.T.astype(_bf))
    wvT = _tile_kxm(wv.T.astype(_bf))

    in_maps = []
    for c in range(N_CORES):
        sl = slice(c * 256, (c + 1) * 256)
        tsl = slice(c * SH, (c + 1) * SH)
        in_maps.append({
            "hsT": hsT,
            "hskv": _tile_kxm(hsT[:, tsl]),
            "wq": _tile_kxm(wq[sl, :].T.astype(_bf)),
            "wk": wkT,
            "wv": wvT,
            "wo": _tile_kxm(wo[sl, :].T.astype(_bf)),
            "cosT": cosT,
            "sinT": sinT,
            "coskv": np.ascontiguousarray(cosT[:, tsl]),
            "sinkv": np.ascontiguousarray(sinT[:, tsl]),
        })
    return in_maps


def _run(in_maps, trace=False):
    nc = _build()
    kw = {"trace": True, "trace_cores": list(range(N_CORES))} if trace else {}
    return run_bass_kernel_spmd(nc, in_maps, core_ids=list(range(N_CORES)), **kw)


def _assemble(results):
    cols = [results[c]["out"].T for c in range(N_CORES)]          # [T, 256] each
    full = np.concatenate(cols, axis=1)                           # [T, HID]
    return np.ascontiguousarray(full.reshape(B, S, HID).astype(np.float32))


def kernel(hidden_states, attention_mask, position_ids, wq, wk, wv, wo):
    in_maps = _prepare(hidden_states, position_ids, wq, wk, wv, wo)
    res = _run(in_maps, trace=False)
    return _assemble(res.results)


def run_traced(hidden_states, attention_mask, position_ids, wq, wk, wv, wo):
    """Like kernel(), but also captures a neuron-profile trace.
    Returns (output, BassKernelResults)."""
    in_maps = _prepare(hidden_states, position_ids, wq, wk, wv, wo)
    res = _run(in_maps, trace=True)
    return _assemble(res.results), res


# revision 25
# speedup vs baseline: 1.1725x; 1.1725x over previous
"""Gemma attention (B=2, S=2048, HID=2048, H=8 q-heads, 1 KV head, D=256)
as a Bass/Tile SPMD kernel on 8 TRN2 NeuronCores.

Distribution (tensor-parallel over query heads):
  - core c owns query head c: wq/wo split along the head axis.
  - k/v projection is sharded over tokens (512 tokens/core), then
    AllGathered (k in transposed layout, v in natural layout; a ones
    column on v makes the softmax denominator fall out of the PV matmul).
  - softmax skips the max-subtraction (scores ~ N(0,1); exp is safe in
    fp32) and is computed on the transposed score layout so no transposes
    are needed before the PV matmul.
  - o_proj: per-head attention outputs (transposed [D, T]) are
    AllGathered quarter-by-quarter (pipelined behind attention) to form
    A.T = [H*D, T]; each core computes its own 256-column slice of the
    output, so no AllReduce is needed at all.

DMA ring notes: HWDGE FIFOs are per issuing engine (sync=SP, scalar=ACT)
and a DMA that waits on a collective blocks everything behind it on the
same ring. Streaming loads alternate between the SP and ACT rings for
bandwidth; collective bounce-buffer stores go on the idle GpSimd SWDGE;
collective-result loads go at points where their ring is already drained.

All matmuls run in bf16 with fp32 PSUM accumulation; RoPE cos/sin tables
are precomputed on the host from position_ids.
"""
import numpy as np
import ml_dtypes

import concourse.bass as bass
import concourse.mybir as mybir
import concourse.tile as tile
from concourse import bacc
from concourse.bass_utils import run_bass_kernel_spmd
from concourse.masks import make_identity

B, S, HID = 2, 2048, 2048
H, D = 8, 256
N_CORES = 8
T = B * S              # 4096 tokens total
SH = T // N_CORES      # 512 kv tokens per core
BASE = 10000.0
BF16 = mybir.dt.bfloat16
F32 = mybir.dt.float32
RG = [list(range(N_CORES))]
AF = mybir.ActivationFunctionType
_bf = ml_dtypes.bfloat16

KC = HID // 128        # 16 contraction chunks
SCALE = 1.0 / np.sqrt(D)


class _RingChain:
    """Chains DMA loads per HWDGE ring in emission order so the scheduler
    cannot hoist a collective-dependent load above the streaming loads
    (head-of-line blocking on the ring FIFO)."""

    def __init__(self, nc):
        self.engs = [nc.sync, nc.scalar]
        self.prev = [None, None]

    def dma(self, ring, dst, src):
        from concourse.tile_rust import add_dep_helper
        li = self.engs[ring].dma_start(dst, src)
        if self.prev[ring] is not None:
            add_dep_helper(li.ins, self.prev[ring].ins, sync=False,
                           reason="ring FIFO order")
        self.prev[ring] = li
        return li


def _attention_block(nc, psA, psB, psT, ptp, work, kt_sb, v_sb, q_sb, o_sb,
                     ident, b, tb):
    """Scores^T -> exp -> PV (ones-augmented) -> normalize -> transpose."""
    tq = b * S + tb * 512
    pt = ptp.tile([128, 16 * 512], BF16, tag="pt", name=f"pt{b}_{tb}")
    for uc in range(16):
        sp = psA.tile([128, 512], F32, tag="mm512", name=f"sp{b}_{tb}_{uc}")
        for dc in range(2):
            x = (b * 4 + uc // 4) * 2 + dc
            nc.tensor.matmul(
                sp[:],
                lhsT=kt_sb[:, x * 512 + (uc % 4) * 128:x * 512 + (uc % 4 + 1) * 128],
                rhs=q_sb[dc][:, tq:tq + 512],
                start=(dc == 0), stop=(dc == 1))
        nc.scalar.activation(pt[:, uc * 512:(uc + 1) * 512], sp[:],
                             AF.Exp, scale=float(SCALE))
    for ts in range(4):
        av = psB.tile([128, 257], F32, tag="acc", name=f"av{b}_{tb}_{ts}")
        for uc in range(16):
            nc.tensor.matmul(
                av[:],
                lhsT=pt[:, uc * 512 + ts * 128:uc * 512 + (ts + 1) * 128],
                rhs=v_sb[:, (b * 16 + uc) * 257:(b * 16 + uc + 1) * 257],
                start=(uc == 0), stop=(uc == 15))
        recip = work.tile([128, 1], F32, tag="recip", name=f"rc{b}_{tb}_{ts}")
        nc.vector.reciprocal(recip[:], av[:, 256:257])
        onat = work.tile([128, 256], BF16, tag="onat", name=f"on{b}_{tb}_{ts}")
        nc.scalar.activation(onat[:], av[:, 0:256], AF.Copy, scale=recip[:])
        for dcc in range(2):
            trp = psT.tile([128, 128], BF16, tag="tr", name=f"tr{b}_{tb}_{ts}_{dcc}")
            nc.tensor.transpose(trp[:], onat[:, dcc * 128:(dcc + 1) * 128], ident[:])
            nc.vector.tensor_copy(
                o_sb[b][dcc][:, tb * 512 + ts * 128:tb * 512 + (ts + 1) * 128],
                trp[:])


def _oag_start(nc, dram, o_sb, oag, tok0, width):
    """Store A^T for tokens [tok0, tok0+width) to DRAM and AllGather it."""
    b, off = tok0 // S, tok0 % S
    oin = dram.tile([256, width], BF16, name=f"oag_in{tok0}")
    oout = dram.tile([2048, width], BF16, addr_space="Shared",
                     name=f"oag_out{tok0}")
    for dcc in range(2):
        nc.gpsimd.dma_start(oin[dcc * 128:(dcc + 1) * 128, :],
                            o_sb[b][dcc][:, off:off + width])
    nc.gpsimd.collective_compute(
        "AllGather", mybir.AluOpType.bypass, replica_groups=RG,
        ins=[oin[:]], outs=[oout[:]])
    oag.append((oout, tok0, width))


def _oproj_piece(nc, psA, stB, work, wo_sb, out, oag, q, chain):
    """o_proj for one gathered A^T piece (width 512 or 1024 tokens)."""
    oout, tok0, width = oag[q]
    at = stB.tile([128, 16 * 1024], BF16, tag="at", name=f"at{q}")
    for ji in range(4):
        chain.dma(
            0,
            at[:, ji * width * 4:(ji + 1) * width * 4]
                .rearrange("p (x t) -> p x t", x=4),
            oout[ji * 512:(ji + 1) * 512, :].rearrange("(x p) t -> p x t", p=128))
    # transposed o_proj: out^T[c, t] = sum_j woT[j, c] * A^T[j, t] — N=512
    # moving dim, half the matmul instructions; host untransposes.
    for h in range(width // 512):
        for cc in range(2):
            op = psA.tile([128, 512], F32, tag="mm512", name=f"op{q}_{h}_{cc}")
            for jc in range(16):
                nc.tensor.matmul(
                    op[:],
                    lhsT=wo_sb[:, jc * 256 + cc * 128:jc * 256 + (cc + 1) * 128],
                    rhs=at[:, jc * width + h * 512:jc * width + (h + 1) * 512],
                    start=(jc == 0), stop=(jc == KC - 1))
            osb = work.tile([128, 512], F32, tag="osb", name=f"os{q}_{h}_{cc}")
            nc.scalar.copy(osb[:], op[:])
            col = tok0 + h * 512
            nc.scalar.dma_start(out[cc * 128:(cc + 1) * 128, col:col + 512], osb[:])


def _body(nc, tc, io):
    hsT, hskv = io["hsT"], io["hskv"]
    wq, wk, wv, wo = io["wq"], io["wk"], io["wv"], io["wo"]
    cosT, sinT = io["cosT"], io["sinT"]
    coskv, sinkv = io["coskv"], io["sinkv"]
    out = io["out"]

    with (
        tc.tile_pool(name="const", bufs=1) as constp,
        tc.tile_pool(name="pers", bufs=1) as pers,
        tc.tile_pool(name="work", bufs=2) as work,
        tc.tile_pool(name="dram", bufs=1, space="DRAM") as dram,
    ):
        ph3_cm = tc.tile_pool(name="ph3", bufs=1)
        ph3 = ph3_cm.__enter__()
        phcs_cm = tc.tile_pool(name="phcs", bufs=1)
        phcs = phcs_cm.__enter__()
        ph12_cm = tc.tile_pool(name="ph12", bufs=1)
        ph12 = ph12_cm.__enter__()
        psA_cm = tc.tile_pool(name="psA", bufs=4, space="PSUM")
        psA = psA_cm.__enter__()
        psB_cm = tc.tile_pool(name="psB", bufs=3, space="PSUM")
        psB = psB_cm.__enter__()
        psT_cm = tc.tile_pool(name="psT", bufs=1, space="PSUM")
        psT = psT_cm.__enter__()

        # ---- kv-critical loads first, split across both HWDGE rings ----
        chain = _RingChain(nc)
        wk_sb = constp.tile([128, KC * 256], BF16, name="wk_sb")
        wv_sb = constp.tile([128, KC * 256], BF16, name="wv_sb")
        hskv_sb = ph12.tile([128, KC * SH], BF16, name="hskv_sb")
        chain.dma(0, wk_sb[:, 0:2048], wk[:, 0:2048])
        chain.dma(1, wv_sb[:, 0:2048], wv[:, 0:2048])
        for h in range(4):
            chain.dma(h % 2, hskv_sb[:, h * 2048:(h + 1) * 2048],
                      hskv[:, h * 2048:(h + 1) * 2048])
        chain.dma(0, wk_sb[:, 2048:4096], wk[:, 2048:4096])
        chain.dma(1, wv_sb[:, 2048:4096], wv[:, 2048:4096])
        coskv_sb = constp.tile([128, SH], BF16, name="coskv_sb")
        chain.dma(1, coskv_sb[:], coskv[:])
        sinkv_sb = constp.tile([128, SH], BF16, name="sinkv_sb")
        chain.dma(1, sinkv_sb[:], sinkv[:])
        wq_sb = constp.tile([128, KC * 256], BF16, name="wq_sb")
        chain.dma(0, wq_sb[:], wq[:])
        cosT_sb = phcs.tile([128, T], BF16, name="cosT_sb")
        chain.dma(1, cosT_sb[:], cosT[:])
        sinT_sb = phcs.tile([128, T], BF16, name="sinT_sb")
        chain.dma(1, sinT_sb[:], sinT[:])
        ident = constp.tile([128, 128], BF16, name="ident")
        make_identity(nc, ident[:])

        # ---- DRAM comm buffers (k AG first so scores unblock earliest) ----
        kag_in = dram.tile([256, SH], BF16, name="kag_in")
        kag_out = dram.tile([256 * N_CORES, SH], BF16, addr_space="Shared",
                            name="kag_out")
        vag_in = dram.tile([SH, 257], BF16, name="vag_in")
        vag_out = dram.tile([T, 257], BF16, addr_space="Shared", name="vag_out")

        # ---- phase 1: kv projection on this core's 512 tokens ----
        kps = []
        for dc in range(2):
            kp = psA.tile([128, SH], F32, tag="mm512", name=f"kp{dc}")
            for kc in range(KC):
                nc.tensor.matmul(
                    kp[:],
                    lhsT=wk_sb[:, kc * 256 + dc * 128:kc * 256 + (dc + 1) * 128],
                    rhs=hskv_sb[:, kc * SH:(kc + 1) * SH],
                    start=(kc == 0), stop=(kc == KC - 1))
            kps.append(kp)
        for dc in range(2):
            ra = work.tile([128, SH], F32, tag="ropeA", name=f"kra{dc}")
            rb = work.tile([128, SH], F32, tag="ropeB", bufs=1, name=f"krb{dc}")
            kst = work.tile([128, SH], BF16, tag="kst", bufs=1, name=f"kst{dc}")
            if dc == 0:
                nc.vector.tensor_mul(ra[:], kps[0][:], coskv_sb[:])
                nc.vector.tensor_mul(rb[:], kps[1][:], sinkv_sb[:])
                nc.vector.tensor_sub(kst[:], ra[:], rb[:])
            else:
                nc.vector.tensor_mul(ra[:], kps[1][:], coskv_sb[:])
                nc.vector.tensor_mul(rb[:], kps[0][:], sinkv_sb[:])
                nc.vector.tensor_add(kst[:], ra[:], rb[:])
            nc.gpsimd.dma_start(kag_in[dc * 128:(dc + 1) * 128, :], kst[:])
        nc.gpsimd.collective_compute(
            "AllGather", mybir.AluOpType.bypass, replica_groups=RG,
            ins=[kag_in[:]], outs=[kag_out[:]])
        for uu in range(4):
            vp = psB.tile([128, 257], F32, tag="acc", name=f"vp{uu}")
            for kc in range(KC):
                nc.tensor.matmul(
                    vp[:, 0:256],
                    lhsT=hskv_sb[:, kc * SH + uu * 128:kc * SH + (uu + 1) * 128],
                    rhs=wv_sb[:, kc * 256:(kc + 1) * 256],
                    start=(kc == 0), stop=(kc == KC - 1))
            vst = work.tile([128, 257], BF16, tag="vst", bufs=1, name=f"vst{uu}")
            nc.scalar.copy(vst[:, 0:256], vp[:, 0:256])
            nc.vector.memset(vst[:, 256:257], 1.0)
            nc.gpsimd.dma_start(vag_in[uu * 128:(uu + 1) * 128, :], vst[:])
        nc.gpsimd.collective_compute(
            "AllGather", mybir.AluOpType.bypass, replica_groups=RG,
            ins=[vag_in[:]], outs=[vag_out[:]])

        # ---- phase 2: q projection + RoPE; batch-0 tiles also produce
        # LOCAL k/v for batch 0 (attention b0 then has no collective dep;
        # the kv AllGather only matters for batch 1, hiding the CC floor).
        q_sb = [ph3.tile([128, T], BF16, name=f"q{dc}_sb") for dc in range(2)]
        kt_sb = ph3.tile([128, 16 * 512], BF16, name="kt_sb")
        v_sb = ph3.tile([128, 32 * 257], BF16, name="v_sb")

        def q_tile(tb):
            hst = ph12.tile([128, KC * 512], BF16, tag="hst", bufs=4,
                            name=f"hst{tb}")
            chain.dma(
                tb % 2,
                hst.rearrange("p (x t) -> p x t", x=KC),
                hsT[:, tb * 512:(tb + 1) * 512].rearrange("(x p) t -> p x t", p=128))
            qps = []
            for dc in range(2):
                qp = psA.tile([128, 512], F32, tag="mm512", name=f"qp{tb}_{dc}")
                for kc in range(KC):
                    nc.tensor.matmul(
                        qp[:],
                        lhsT=wq_sb[:, kc * 256 + dc * 128:kc * 256 + (dc + 1) * 128],
                        rhs=hst[:, kc * 512:(kc + 1) * 512],
                        start=(kc == 0), stop=(kc == KC - 1))
                qps.append(qp)
            cs = cosT_sb[:, tb * 512:(tb + 1) * 512]
            sn = sinT_sb[:, tb * 512:(tb + 1) * 512]
            for dc in range(2):
                ra = work.tile([128, 512], F32, tag="ropeA", name=f"qra{tb}_{dc}")
                rb = work.tile([128, 512], F32, tag="ropeB", bufs=1, name=f"qrb{tb}_{dc}")
                if dc == 0:
                    nc.vector.tensor_mul(ra[:], qps[0][:], cs)
                    nc.vector.tensor_mul(rb[:], qps[1][:], sn)
                    nc.vector.tensor_sub(q_sb[0][:, tb * 512:(tb + 1) * 512], ra[:], rb[:])
                else:
                    nc.vector.tensor_mul(ra[:], qps[1][:], cs)
                    nc.vector.tensor_mul(rb[:], qps[0][:], sn)
                    nc.vector.tensor_add(q_sb[1][:, tb * 512:(tb + 1) * 512], ra[:], rb[:])

        for tb in range(T // 512):
            q_tile(tb)
        ph12_cm.__exit__(None, None, None)
        phcs_cm.__exit__(None, None, None)

        ptp_cm = tc.tile_pool(name="ptp", bufs=2)
        ptp = ptp_cm.__enter__()
        stB_cm = tc.tile_pool(name="stB", bufs=2)
        stB = stB_cm.__enter__()

        # wo needed from the first o_proj piece (~50% into the kernel)
        wo_sb = constp.tile([128, KC * 256], BF16, name="wo_sb")
        chain.dma(0, wo_sb[:], wo[:])

        # ---- phases 2b/3/4 interleaved ----
        o_sb = [[pers.tile([128, S], BF16, name=f"o{b}_{dcc}_sb")
                 for dcc in range(2)] for b in range(2)]
        oag = []
        ab = lambda b, tb: _attention_block(nc, psA, psB, psT, ptp, work, kt_sb,
                                            v_sb, q_sb, o_sb, ident, b, tb)
        op = lambda q: _oproj_piece(nc, psA, stB, work, wo_sb, out, oag, q,
                                    chain)
        # gathered k/v (all ranks; chained on sync ring)
        for r in range(N_CORES):
            chain.dma(
                0, kt_sb[:, r * 1024:(r + 1) * 1024].rearrange("p (x u) -> p x u", x=2),
                kag_out[r * 256:(r + 1) * 256, :].rearrange("(x p) u -> p x u", p=128))
        for r in range(N_CORES):
            chain.dma(
                0, v_sb[:, r * 4 * 257:(r + 1) * 4 * 257].rearrange("p (x d) -> p x d", x=4),
                vag_out[r * 512:(r + 1) * 512, :].rearrange("(x p) d -> p x d", p=128))
        ab(0, 0); ab(0, 1)
        _oag_start(nc, dram, o_sb, oag, 0, 1024)
        ab(0, 2); ab(0, 3)
        _oag_start(nc, dram, o_sb, oag, 1024, 1024)
        op(0)
        ab(1, 0); ab(1, 1)
        _oag_start(nc, dram, o_sb, oag, 2048, 1024)
        op(1)
        ab(1, 2)
        _oag_start(nc, dram, o_sb, oag, 3072, 512)
        ab(1, 3)
        _oag_start(nc, dram, o_sb, oag, 3584, 512)
        op(2)
        op(3)
        op(4)

        stB_cm.__exit__(None, None, None)
        ptp_cm.__exit__(None, None, None)
        ph3_cm.__exit__(None, None, None)
        psT_cm.__exit__(None, None, None)
        psB_cm.__exit__(None, None, None)
        psA_cm.__exit__(None, None, None)


_NC_CACHE = {}


def _build():
    if "nc" in _NC_CACHE:
        return _NC_CACHE["nc"]
    nc = bacc.Bacc("TRN2", target_bir_lowering=False, debug=False,
                   enable_asserts=False, num_devices=N_CORES)
    io = {}
    io["hsT"] = nc.dram_tensor("hsT", [HID, T], BF16, kind="ExternalInput").ap()
    io["hskv"] = nc.dram_tensor("hskv", [128, KC * SH], BF16, kind="ExternalInput").ap()
    for w in ("wq", "wk", "wv", "wo"):
        io[w] = nc.dram_tensor(w, [128, KC * 256], BF16, kind="ExternalInput").ap()
    io["cosT"] = nc.dram_tensor("cosT", [128, T], BF16, kind="ExternalInput").ap()
    io["sinT"] = nc.dram_tensor("sinT", [128, T], BF16, kind="ExternalInput").ap()
    io["coskv"] = nc.dram_tensor("coskv", [128, SH], BF16, kind="ExternalInput").ap()
    io["sinkv"] = nc.dram_tensor("sinkv", [128, SH], BF16, kind="ExternalInput").ap()
    io["out"] = nc.dram_tensor("out", [256, T], F32, kind="ExternalOutput").ap()
    with tile.TileContext(nc) as tc:
        _body(nc, tc, io)
    nc.compile()
    _NC_CACHE["nc"] = nc
    return nc


def _tile_kxm(a):
    """[HID, M] -> [128, KC*M] with column block kc holding rows kc*128..+128."""
    hid, m = a.shape
    return np.ascontiguousarray(
        a.reshape(hid // 128, 128, m).transpose(1, 0, 2).reshape(128, -1))


def _prepare(hidden_states, position_ids, wq, wk, wv, wo):
    hs = np.asarray(hidden_states, dtype=np.float32).reshape(T, HID)
    hsT = np.ascontiguousarray(hs.T).astype(_bf)                 # [HID, T]

    inv_freq = 1.0 / (BASE ** (np.arange(0, D, 2, dtype=np.float64) / D))
    pos = np.asarray(position_ids).astype(np.float64).reshape(T)
    ang = inv_freq[:, None] * pos[None, :]                        # [128, T]
    cosT = np.cos(ang).astype(_bf)
    sinT = np.sin(ang).astype(_bf)

    wq = np.asarray(wq, dtype=np.float32)
    wk = np.asarray(wk, dtype=np.float32)
    wv = np.asarray(wv, dtype=np.float32)
    wo = np.asarray(wo, dtype=np.float32)
    wkT = _tile_kxm(wk.T.astype(_bf))
    wvT = _tile_kxm(wv.T.astype(_bf))

    in_maps = []
    for c in range(N_CORES):
        sl = slice(c * 256, (c + 1) * 256)
        tsl = slice(c * SH, (c + 1) * SH)
        in_maps.append({
            "hsT": hsT,
            "hskv": _tile_kxm(hsT[:, tsl]),
            "wq": _tile_kxm(wq[sl, :].T.astype(_bf)),
            "wk": wkT,
            "wv": wvT,
            "wo": _tile_kxm(wo[sl, :].T.astype(_bf)),
            "cosT": cosT,
            "sinT": sinT,
            "coskv": np.ascontiguousarray(cosT[:, tsl]),
            "sinkv": np.ascontiguousarray(sinT[:, tsl]),
        })
    return in_maps


def _run(in_maps, trace=False):
    nc = _build()
    kw = {"trace": True, "trace_cores": list(range(N_CORES))} if trace else {}
    return run_bass_kernel_spmd(nc, in_maps, core_ids=list(range(N_CORES)), **kw)


def _assemble(results):
    cols = [results[c]["out"].T for c in range(N_CORES)]          # [T, 256] each
    full = np.concatenate(cols, axis=1)                           # [T, HID]
    return np.ascontiguousarray(full.reshape(B, S, HID).astype(np.float32))


def kernel(hidden_states, attention_mask, position_ids, wq, wk, wv, wo):
    in_maps = _prepare(hidden_states, position_ids, wq, wk, wv, wo)
    res = _run(in_maps, trace=False)
    return _assemble(res.results)


def run_traced(hidden_states, attention_mask, position_ids, wq, wk, wv, wo):
    """Like kernel(), but also captures a neuron-profile trace.
    Returns (output, BassKernelResults)."""
    in_maps = _prepare(hidden_states, position_ids, wq, wk, wv, wo)
    res = _run(in_maps, trace=True)
    return _assemble(res.results), res
